# revision 8
# baseline (speedup 1.0000x reference)
"""Trainium2 Bass kernel for nn_Former_Mobile (mobile-former style cross-attention).

Computation (per batch item n):
    kv   = relu6(global_feature @ W_kv^T + b_kv)        # [m=8, 2c]
    K, V = kv[:, :c], kv[:, c:]                         # [8, c=384]
    q    = x reshaped [hw=3136, c]
    attn = softmax(q @ K^T)                             # [hw, 8]
    out  = (attn @ V) reshaped back + x                 # [c, hw]

Sharding: data-parallel over batch n across 8 NeuronCores (4 items each);
W_kv/b_kv replicated (bias folded into an extra contraction row host-side).

Matmul operands use float32r (PE relaxed-precision fp32: bf16-class speed,
~1e-4 relative rounding), accumulation in fp32 PSUM. Exact fp32 matmul on
TRN2 runs 4-8x slower per column (hi/lo dual pass at reduced rate), which
makes an fp32-exact kernel ~3x off the memory roofline; f32r recovers it.

Per-core device pipeline:
  phase 0: kv = gft-chunks @ wt-chunks (PE, psum accum) -> relu6 -> K^T via
           PE transpose (mm1 weights), per-n V rows (mm2 weights).
  per n:
    mm1   scoresT[8, hw-tile] = K^T(lhsT, 8 cols) @ x-chunk(rhs, K=128
          full-rate streaming), psum-accumulated over 3 c-chunks.
    T1    PE-transposes scoresT 128-blocks into scores[hw_p, m] psum macros
          (transpose-mode has fast weight load; f32r at ~1 cyc/col).
    softmax along free dim: DVE grouped reduce_max(negate) -> add broadcast
          -> ACT exp -> DVE grouped reduce_sum -> reciprocal -> mul.
    T2    PE-transposes attn tiles back into attnT[8, hw].
    mm2   out^T[c_p, hw-tile] = V(lhsT) @ attnT(rhs), single K=8 matmul.
    DVE residual add (psum + x -> sbuf), contiguous DMA out.
"""

import sys

if "/opt/trn_rl_repo" not in sys.path:
    sys.path.insert(0, "/opt/trn_rl_repo")

import numpy as np

N, C, H, W = 32, 384, 56, 56
HW = H * W                      # 3136
M, D = 8, 768
N_CORES = 8
N_LOC = N // N_CORES            # 4 batch items per core
NM = N_LOC * M                  # 32 kv rows per core
D1P = 896                       # 768 + bias row, zero-padded to 7*128
KC = C // 128                   # 3 contraction chunks over c
P = 128

# hw subtiles (128 wide) for the softmax layout: 24 x 128 + 1 x 64
HWT = [128] * 24 + [64]
# macro groups of subtiles sharing one psum bank + one softmax pass
MACROS = [(0, 16), (16, 8), (24, 1)]
# scoresT hw tiles (one psum bank each)
HWT2 = [512] * 6 + [64]

_cache = {}
last_results = None


def _build():
    from concourse import bacc, tile, mybir
    from concourse.masks import make_identity

    f32 = mybir.dt.float32
    f32r = mybir.dt.float32r
    Alu = mybir.AluOpType
    Act = mybir.ActivationFunctionType
    PSUM = tile.bass.MemorySpace.PSUM

    nc = bacc.Bacc("TRN2", target_bir_lowering=False, debug=False,
                   num_devices=N_CORES)

    xs_d = nc.dram_tensor("xs", [N_LOC, C, HW], f32r, kind="ExternalInput")
    gft_d = nc.dram_tensor("gft", [D1P, NM], f32r, kind="ExternalInput")
    wt_d = nc.dram_tensor("wt", [D1P, D], f32r, kind="ExternalInput")
    out_d = nc.dram_tensor("out", [N_LOC, C, HW], f32, kind="ExternalOutput")

    with tile.TileContext(nc) as tc:
        with tc.tile_pool(name="const", bufs=1) as const:
            ident = const.tile([P, P], f32, tag="ident")
            make_identity(nc, ident[:, :])
            identr = const.tile([P, P], f32r, tag="identr")
            nc.vector.tensor_copy(identr[:, :], ident[:, :])

            K_sb = const.tile([NM, C], f32r, tag="K_sb")
            V_n = [const.tile([M, C], f32r, tag=f"V{n}", name=f"V{n}")
                   for n in range(N_LOC)]
            KT = [const.tile([P, NM], f32r, tag=f"KT{kc}", name=f"KT{kc}")
                  for kc in range(KC)]

            with tc.tile_pool(name="wtp", bufs=1) as wtp, \
                 tc.tile_pool(name="psum0", bufs=1, space=PSUM) as psum0:
                wt_sb = []
                gft_sb = []
                for i in range(7):
                    w = wtp.tile([P, D], f32r, tag=f"wt{i}", name=f"wt{i}")
                    nc.sync.dma_start(w[:, :], wt_d.ap()[i * P:(i + 1) * P, :])
                    wt_sb.append(w)
                    g = const.tile([P, NM], f32r, tag=f"gft{i}", name=f"gft{i}")
                    nc.sync.dma_start(g[:, :],
                                      gft_d.ap()[i * P:(i + 1) * P, :])
                    gft_sb.append(g)
                kvK = psum0.tile([NM, C], f32, tag="kvK")
                for i in range(7):
                    nc.tensor.matmul(
                        kvK[:, :], gft_sb[i][:, :], wt_sb[i][:, :C],
                        start=(i == 0), stop=(i == 6))
                nc.vector.tensor_scalar(K_sb[:, :], kvK[:, :], 0.0, 6.0,
                                        op0=Alu.max, op1=Alu.min)
                # V per batch item at partition 0 (engine APs can't start at
                # partition 8/16/24), via lhsT free-dim slices of gft
                for n in range(N_LOC):
                    kvV = psum0.tile([M, C], f32, tag=f"kvV{n}",
                                     name=f"kvV{n}")
                    for i in range(7):
                        nc.tensor.matmul(
                            kvV[:, :], gft_sb[i][:, n * M:(n + 1) * M],
                            wt_sb[i][:, C:2 * C],
                            start=(i == 0), stop=(i == 6))
                    nc.vector.tensor_scalar(V_n[n][:, :], kvV[:, :],
                                            0.0, 6.0, op0=Alu.max, op1=Alu.min)
                for kc in range(KC):
                    ktp = psum0.tile([P, NM], f32r, tag="ktp")
                    nc.tensor.transpose(ktp[:, :],
                                        K_sb[:, kc * P:(kc + 1) * P],
                                        identr[:NM, :NM])
                    nc.scalar.copy(KT[kc][:, :], ktp[:, :])

            with (
                tc.tile_pool(name="xp", bufs=10) as xp,
                tc.tile_pool(name="sm", bufs=4) as sm,
                tc.tile_pool(name="sc8", bufs=6) as sc8,
                tc.tile_pool(name="aTp", bufs=2) as aTpool,
                tc.tile_pool(name="op", bufs=2) as op,
                tc.tile_pool(name="p8", bufs=3, space=PSUM) as p8,
                tc.tile_pool(name="ps_s", bufs=2, space=PSUM) as ps_s,
                tc.tile_pool(name="ps_o", bufs=3, space=PSUM) as ps_o,
            ):
                for n in range(N_LOC):
                    xc = []
                    for kc in range(KC):
                        t = xp.tile([P, HW], f32r, tag="x", name="xtile")
                        nc.sync.dma_start(
                            t[:, :], xs_d.ap()[n, kc * P:(kc + 1) * P, :])
                        xc.append(t)

                    # mm1: scoresT[8, hw] tiles, x streaming at K=128
                    scT = []
                    for t5, w5 in enumerate(HWT2):
                        pst = p8.tile([M, 512], f32, tag="b8", name="pst")
                        for kc in range(KC):
                            nc.tensor.matmul(
                                pst[:, :w5],
                                KT[kc][:, n * M:(n + 1) * M],
                                xc[kc][:, t5 * 512:t5 * 512 + w5],
                                start=(kc == 0), stop=(kc == KC - 1))
                        st = sc8.tile([M, 512], f32r, tag="scT_sb",
                                      name="scT_sb")
                        nc.scalar.copy(st[:, :w5], pst[:, :w5])
                        scT.append(st)

                    aT = aTpool.tile([M, HW], f32r, tag="aT")

                    for ms, G in MACROS:
                        FD = M * G
                        pmac = HWT[ms]
                        ps = ps_s.tile([P, FD], f32r, tag="s")
                        # T1: scoresT 128-blocks -> scores[hw_p, m] slices
                        for jj in range(G):
                            j = ms + jj
                            pj = HWT[j]
                            nc.tensor.transpose(
                                ps[:pj, jj * M:(jj + 1) * M],
                                scT[j // 4][:, (j % 4) * P:(j % 4) * P + pj],
                                identr[:M, :M])

                        psf = ps[:pmac, :].bitcast(f32)
                        ps3 = psf.rearrange("p (g m) -> p g m", m=M)
                        nmx = sm.tile([P, G], f32, tag="nmx")
                        nc.vector.tensor_reduce(nmx[:pmac, :], ps3,
                                                axis=mybir.AxisListType.X,
                                                op=Alu.max, negate=True)
                        nmx_b = nmx[:pmac, :].unsqueeze(-1).broadcast_to(
                            [pmac, G, M])
                        e = sm.tile([P, FD], f32, tag="e")
                        e3 = e[:pmac, :].rearrange("p (g m) -> p g m", m=M)
                        nc.vector.tensor_add(e3, ps3, nmx_b)
                        nc.scalar.activation(e[:pmac, :], e[:pmac, :], Act.Exp)
                        den = sm.tile([P, G], f32, tag="den")
                        nc.vector.tensor_reduce(den[:pmac, :], e3,
                                                axis=mybir.AxisListType.X,
                                                op=Alu.add)
                        r = sm.tile([P, G], f32, tag="r")
                        nc.vector.reciprocal(r[:pmac, :], den[:pmac, :])
                        r_b = r[:pmac, :].unsqueeze(-1).broadcast_to(
                            [pmac, G, M])
                        attn = sm.tile([P, FD], f32r, tag="attn")
                        a3 = attn[:pmac, :].rearrange("p (g m) -> p g m", m=M)
                        nc.vector.tensor_mul(a3, e3, r_b)

                        # T2: attn subtiles -> attnT[8, hw], packed 4/bank
                        for pk in range(0, G, 4):
                            cnt = min(4, G - pk)
                            width = sum(HWT[ms + pk + q] for q in range(cnt))
                            pt = p8.tile([M, 512], f32r, tag="b8", name="pt")
                            for q in range(cnt):
                                jj = pk + q
                                pj = HWT[ms + jj]
                                nc.tensor.transpose(
                                    pt[:, q * P:q * P + pj],
                                    attn[:pj, jj * M:(jj + 1) * M],
                                    identr[:pj, :pj])
                            nc.scalar.copy(
                                aT[:, (ms + pk) * P:(ms + pk) * P + width],
                                pt[:, :width])

                    # mm2 + residual + store
                    for kc in range(KC):
                        osb = op.tile([P, HW], f32, tag="o")
                        for t7 in range(7):
                            po = ps_o.tile([P, 448], f32, tag="po")
                            nc.tensor.matmul(
                                po[:, :],
                                V_n[n][:, kc * P:(kc + 1) * P],
                                aT[:, t7 * 448:(t7 + 1) * 448],
                                start=True, stop=True)
                            nc.vector.tensor_add(
                                osb[:, t7 * 448:(t7 + 1) * 448], po[:, :],
                                xc[kc][:, t7 * 448:(t7 + 1) * 448].bitcast(f32))
                        nc.sync.dma_start(
                            out_d.ap()[n, kc * P:(kc + 1) * P, :], osb[:, :])

    nc.compile()
    return nc


def get_nc():
    if "nc" not in _cache:
        _cache["nc"] = _build()
    return _cache["nc"]


def make_in_maps(x, global_feature, W_kv, b_kv):
    x = np.ascontiguousarray(np.asarray(x, np.float32).reshape(N, C, HW))
    wt = np.zeros((D1P, D), np.float32)
    wt[:D] = np.asarray(W_kv, np.float32).T
    wt[D] = np.asarray(b_kv, np.float32)
    gf = np.asarray(global_feature, np.float32)
    in_maps = []
    for i in range(N_CORES):
        gfl = gf[i * N_LOC:(i + 1) * N_LOC].reshape(NM, D)
        gft = np.zeros((D1P, NM), np.float32)
        gft[:D] = gfl.T
        gft[D] = 1.0
        in_maps.append({
            "xs": np.ascontiguousarray(x[i * N_LOC:(i + 1) * N_LOC]),
            "gft": gft,
            "wt": wt,
        })
    return in_maps


def kernel(x, global_feature, W_kv, b_kv, trace=False):
    global last_results
    from concourse.bass_utils import run_bass_kernel_spmd

    nc = get_nc()
    in_maps = make_in_maps(x, global_feature, W_kv, b_kv)
    res = run_bass_kernel_spmd(nc, in_maps, core_ids=list(range(N_CORES)),
                               trace=trace)
    last_results = res
    out = np.concatenate([res.results[i]["out"][None] for i in range(N_CORES)],
                         axis=0)
    return out.reshape(N, C, H, W).astype(np.float32)


# revision 10
# speedup vs baseline: 1.0846x; 1.0846x over previous
"""Trainium2 Bass kernel for nn_Former_Mobile (mobile-former style cross-attention).

Computation (per batch item n):
    kv   = relu6(global_feature @ W_kv^T + b_kv)        # [m=8, 2c]
    K, V = kv[:, :c], kv[:, c:]                         # [8, c=384]
    q    = x reshaped [hw=3136, c]
    attn = softmax(q @ K^T)                             # [hw, 8]
    out  = (attn @ V) reshaped back + x                 # [c, hw]

Sharding: data-parallel over batch n across 8 NeuronCores (4 items each);
W_kv/b_kv replicated (bias folded into an extra contraction row host-side).

Matmul operands use float32r (PE relaxed-precision fp32: bf16-class speed,
~1e-4 relative rounding), accumulation in fp32 PSUM. Exact fp32 matmul on
TRN2 runs 4-8x slower per column (hi/lo dual pass at reduced rate), which
makes an fp32-exact kernel ~3x off the memory roofline; f32r recovers it.

Per-core device pipeline:
  phase 0: kv = gft-chunks @ wt-chunks (PE, psum accum) -> relu6 -> K^T via
           PE transpose (mm1 weights), per-n V rows (mm2 weights).
  per n:
    mm1   scoresT[8, hw-tile] = K^T(lhsT, 8 cols) @ x-chunk(rhs, K=128
          full-rate streaming), psum-accumulated over 3 c-chunks.
    T1    PE-transposes scoresT 128-blocks into scores[hw_p, m] psum macros
          (transpose-mode has fast weight load; f32r at ~1 cyc/col).
    softmax along free dim: DVE grouped reduce_max(negate) -> add broadcast
          -> ACT exp -> DVE grouped reduce_sum -> reciprocal -> mul.
    T2    PE-transposes attn tiles back into attnT[8, hw].
    mm2   out^T[c_p, hw-tile] = V(lhsT) @ attnT(rhs), single K=8 matmul.
    DVE residual add (psum + x -> sbuf), contiguous DMA out.
"""

import sys

if "/opt/trn_rl_repo" not in sys.path:
    sys.path.insert(0, "/opt/trn_rl_repo")

import numpy as np

N, C, H, W = 32, 384, 56, 56
HW = H * W                      # 3136
M, D = 8, 768
N_CORES = 8
N_LOC = N // N_CORES            # 4 batch items per core
NM = N_LOC * M                  # 32 kv rows per core
D1P = 896                       # 768 + bias row, zero-padded to 7*128
KC = C // 128                   # 3 contraction chunks over c
P = 128

# hw subtiles (128 wide) for the softmax layout: 24 x 128 + 1 x 64
HWT = [128] * 24 + [64]
# macro groups of subtiles sharing one psum bank + one softmax pass
MACROS = [(0, 16), (16, 8), (24, 1)]
# scoresT hw tiles (one psum bank each)
HWT2 = [448] * 7
XA = 1792                       # x chunk split: [0,1792) + [1792,3136)

_cache = {}
last_results = None


def _build():
    from concourse import bacc, tile, mybir
    from concourse.masks import make_identity

    f32 = mybir.dt.float32
    f32r = mybir.dt.float32r
    Alu = mybir.AluOpType
    Act = mybir.ActivationFunctionType
    PSUM = tile.bass.MemorySpace.PSUM

    nc = bacc.Bacc("TRN2", target_bir_lowering=False, debug=False,
                   num_devices=N_CORES)

    xs_d = nc.dram_tensor("xs", [N_LOC, C, HW], f32r, kind="ExternalInput")
    gft_d = nc.dram_tensor("gft", [D1P, NM], f32r, kind="ExternalInput")
    wt_d = nc.dram_tensor("wt", [D1P, D], f32r, kind="ExternalInput")
    out_d = nc.dram_tensor("out", [N_LOC, C, HW], f32, kind="ExternalOutput")

    with tile.TileContext(nc) as tc:
        with tc.tile_pool(name="const", bufs=1) as const:
            ident = const.tile([P, P], f32, tag="ident")
            make_identity(nc, ident[:, :])
            identr = const.tile([P, P], f32r, tag="identr")
            nc.vector.tensor_copy(identr[:, :], ident[:, :])

            K_sb = const.tile([NM, C], f32r, tag="K_sb")
            V_n = [const.tile([M, C], f32r, tag=f"V{n}", name=f"V{n}")
                   for n in range(N_LOC)]
            KT = [const.tile([P, NM], f32r, tag=f"KT{kc}", name=f"KT{kc}")
                  for kc in range(KC)]

            with tc.tile_pool(name="wtp", bufs=1) as wtp, \
                 tc.tile_pool(name="psum0", bufs=1, space=PSUM) as psum0:
                wt_sb = []
                gft_sb = []
                for i in range(7):
                    w = wtp.tile([P, D], f32r, tag=f"wt{i}", name=f"wt{i}")
                    nc.sync.dma_start(w[:, :], wt_d.ap()[i * P:(i + 1) * P, :])
                    wt_sb.append(w)
                    g = const.tile([P, NM], f32r, tag=f"gft{i}", name=f"gft{i}")
                    nc.sync.dma_start(g[:, :],
                                      gft_d.ap()[i * P:(i + 1) * P, :])
                    gft_sb.append(g)
                kvK = psum0.tile([NM, C], f32, tag="kvK")
                for i in range(7):
                    nc.tensor.matmul(
                        kvK[:, :], gft_sb[i][:, :], wt_sb[i][:, :C],
                        start=(i == 0), stop=(i == 6))
                nc.vector.tensor_scalar(K_sb[:, :], kvK[:, :], 0.0, 6.0,
                                        op0=Alu.max, op1=Alu.min)
                # V per batch item at partition 0 (engine APs can't start at
                # partition 8/16/24), via lhsT free-dim slices of gft
                for n in range(N_LOC):
                    kvV = psum0.tile([M, C], f32, tag=f"kvV{n}",
                                     name=f"kvV{n}")
                    for i in range(7):
                        nc.tensor.matmul(
                            kvV[:, :], gft_sb[i][:, n * M:(n + 1) * M],
                            wt_sb[i][:, C:2 * C],
                            start=(i == 0), stop=(i == 6))
                    nc.vector.tensor_scalar(V_n[n][:, :], kvV[:, :],
                                            0.0, 6.0, op0=Alu.max, op1=Alu.min)
                for kc in range(KC):
                    ktp = psum0.tile([P, NM], f32r, tag="ktp")
                    nc.tensor.transpose(ktp[:, :],
                                        K_sb[:, kc * P:(kc + 1) * P],
                                        identr[:NM, :NM])
                    nc.scalar.copy(KT[kc][:, :], ktp[:, :])

            with (
                tc.tile_pool(name="xp", bufs=9) as xp,
                tc.tile_pool(name="sm", bufs=4) as sm,
                tc.tile_pool(name="sc8", bufs=2) as sc8,
                tc.tile_pool(name="aTp", bufs=2) as aTpool,
                tc.tile_pool(name="op", bufs=2) as op,
                tc.tile_pool(name="p8", bufs=3, space=PSUM) as p8,
                tc.tile_pool(name="ps_s", bufs=2, space=PSUM) as ps_s,
                tc.tile_pool(name="ps_o", bufs=3, space=PSUM) as ps_o,
            ):
                for n in range(N_LOC):
                    xc = []
                    for kc in range(KC):
                        ta = xp.tile([P, XA], f32r, tag="xa", name="xa")
                        nc.sync.dma_start(
                            ta[:, :], xs_d.ap()[n, kc * P:(kc + 1) * P, :XA])
                        tb = xp.tile([P, HW - XA], f32r, tag="xb", name="xb")
                        nc.sync.dma_start(
                            tb[:, :], xs_d.ap()[n, kc * P:(kc + 1) * P, XA:])
                        xc.append((ta, tb))

                    def xslice(kc, lo, w):
                        ta, tb = xc[kc]
                        if lo + w <= XA:
                            return ta[:, lo:lo + w]
                        return tb[:, lo - XA:lo - XA + w]

                    # mm1: scoresT[8, hw] tiles, x streaming at K=128
                    scTf = sc8.tile([M, HW], f32r, tag="scT_sb")
                    for t5, w5 in enumerate(HWT2):
                        pst = p8.tile([M, 512], f32, tag="b8", name="pst")
                        for kc in range(KC):
                            nc.tensor.matmul(
                                pst[:, :w5],
                                KT[kc][:, n * M:(n + 1) * M],
                                xslice(kc, t5 * 448, w5),
                                start=(kc == 0), stop=(kc == KC - 1))
                        nc.scalar.copy(scTf[:, t5 * 448:t5 * 448 + w5],
                                       pst[:, :w5])

                    aT = aTpool.tile([M, HW], f32r, tag="aT")

                    for ms, G in MACROS:
                        FD = M * G
                        pmac = HWT[ms]
                        ps = ps_s.tile([P, FD], f32r, tag="s")
                        # T1: scoresT 128-blocks -> scores[hw_p, m] slices
                        for jj in range(G):
                            j = ms + jj
                            pj = HWT[j]
                            nc.tensor.transpose(
                                ps[:pj, jj * M:(jj + 1) * M],
                                scTf[:, j * P:j * P + pj],
                                identr[:M, :M])

                        psf = ps[:pmac, :].bitcast(f32)
                        ps3 = psf.rearrange("p (g m) -> p g m", m=M)
                        nmx = sm.tile([P, G], f32, tag="nmx")
                        nc.vector.tensor_reduce(nmx[:pmac, :], ps3,
                                                axis=mybir.AxisListType.X,
                                                op=Alu.max, negate=True)
                        nmx_b = nmx[:pmac, :].unsqueeze(-1).broadcast_to(
                            [pmac, G, M])
                        e = sm.tile([P, FD], f32, tag="e")
                        e3 = e[:pmac, :].rearrange("p (g m) -> p g m", m=M)
                        nc.vector.tensor_add(e3, ps3, nmx_b)
                        nc.scalar.activation(e[:pmac, :], e[:pmac, :], Act.Exp)
                        den = sm.tile([P, G], f32, tag="den")
                        nc.vector.tensor_reduce(den[:pmac, :], e3,
                                                axis=mybir.AxisListType.X,
                                                op=Alu.add)
                        r = sm.tile([P, G], f32, tag="r")
                        nc.vector.reciprocal(r[:pmac, :], den[:pmac, :])
                        r_b = r[:pmac, :].unsqueeze(-1).broadcast_to(
                            [pmac, G, M])
                        attn = sm.tile([P, FD], f32r, tag="attn")
                        a3 = attn[:pmac, :].rearrange("p (g m) -> p g m", m=M)
                        nc.vector.tensor_mul(a3, e3, r_b)

                        # T2: attn subtiles -> attnT[8, hw], packed 4/bank
                        for pk in range(0, G, 4):
                            cnt = min(4, G - pk)
                            width = sum(HWT[ms + pk + q] for q in range(cnt))
                            pt = p8.tile([M, 512], f32r, tag="b8", name="pt")
                            for q in range(cnt):
                                jj = pk + q
                                pj = HWT[ms + jj]
                                nc.tensor.transpose(
                                    pt[:, q * P:q * P + pj],
                                    attn[:pj, jj * M:(jj + 1) * M],
                                    identr[:pj, :pj])
                            nc.scalar.copy(
                                aT[:, (ms + pk) * P:(ms + pk) * P + width],
                                pt[:, :width])

                    # mm2 + residual + store
                    for kc in range(KC):
                        osb = op.tile([P, HW], f32, tag="o")
                        for t7 in range(7):
                            po = ps_o.tile([P, 448], f32, tag="po")
                            nc.tensor.matmul(
                                po[:, :],
                                V_n[n][:, kc * P:(kc + 1) * P],
                                aT[:, t7 * 448:(t7 + 1) * 448],
                                start=True, stop=True)
                            nc.vector.tensor_add(
                                osb[:, t7 * 448:(t7 + 1) * 448], po[:, :],
                                xslice(kc, t7 * 448, 448).bitcast(f32))
                        nc.scalar.dma_start(
                            out_d.ap()[n, kc * P:(kc + 1) * P, :], osb[:, :])

    nc.compile()
    return nc


def get_nc():
    if "nc" not in _cache:
        _cache["nc"] = _build()
    return _cache["nc"]


def make_in_maps(x, global_feature, W_kv, b_kv):
    x = np.ascontiguousarray(np.asarray(x, np.float32).reshape(N, C, HW))
    wt = np.zeros((D1P, D), np.float32)
    wt[:D] = np.asarray(W_kv, np.float32).T
    wt[D] = np.asarray(b_kv, np.float32)
    gf = np.asarray(global_feature, np.float32)
    in_maps = []
    for i in range(N_CORES):
        gfl = gf[i * N_LOC:(i + 1) * N_LOC].reshape(NM, D)
        gft = np.zeros((D1P, NM), np.float32)
        gft[:D] = gfl.T
        gft[D] = 1.0
        in_maps.append({
            "xs": np.ascontiguousarray(x[i * N_LOC:(i + 1) * N_LOC]),
            "gft": gft,
            "wt": wt,
        })
    return in_maps


def kernel(x, global_feature, W_kv, b_kv, trace=False):
    global last_results
    from concourse.bass_utils import run_bass_kernel_spmd

    nc = get_nc()
    in_maps = make_in_maps(x, global_feature, W_kv, b_kv)
    res = run_bass_kernel_spmd(nc, in_maps, core_ids=list(range(N_CORES)),
                               trace=trace)
    last_results = res
    out = np.concatenate([res.results[i]["out"][None] for i in range(N_CORES)],
                         axis=0)
    return out.reshape(N, C, H, W).astype(np.float32)


# revision 11
# speedup vs baseline: 1.1225x; 1.0350x over previous
"""Trainium2 Bass kernel for nn_Former_Mobile (mobile-former style cross-attention).

Computation (per batch item n):
    kv   = relu6(global_feature @ W_kv^T + b_kv)        # [m=8, 2c]
    K, V = kv[:, :c], kv[:, c:]                         # [8, c=384]
    q    = x reshaped [hw=3136, c]
    attn = softmax(q @ K^T)                             # [hw, 8]
    out  = (attn @ V) reshaped back + x                 # [c, hw]

Sharding: data-parallel over batch n across 8 NeuronCores (4 items each);
W_kv/b_kv replicated (bias folded into an extra contraction row host-side).

Matmul operands use float32r (PE relaxed-precision fp32: bf16-class speed,
~1e-4 relative rounding), accumulation in fp32 PSUM. Exact fp32 matmul on
TRN2 runs 4-8x slower per column (hi/lo dual pass at reduced rate), which
makes an fp32-exact kernel ~3x off the memory roofline; f32r recovers it.

Per-core device pipeline:
  phase 0: kv = gft-chunks @ wt-chunks (PE, psum accum) -> relu6 -> K^T via
           PE transpose (mm1 weights), per-n V rows (mm2 weights).
  per n:
    mm1   scoresT[8, hw-tile] = K^T(lhsT, 8 cols) @ x-chunk(rhs, K=128
          full-rate streaming), psum-accumulated over 3 c-chunks.
    T1    PE-transposes scoresT 128-blocks into scores[hw_p, m] psum macros
          (transpose-mode has fast weight load; f32r at ~1 cyc/col).
    softmax along free dim: DVE grouped reduce_max(negate) -> add broadcast
          -> ACT exp -> DVE grouped reduce_sum -> reciprocal -> mul.
    T2    PE-transposes attn tiles back into attnT[8, hw].
    mm2   out^T[c_p, hw-tile] = V(lhsT) @ attnT(rhs), single K=8 matmul.
    DVE residual add (psum + x -> sbuf), contiguous DMA out.
"""

import sys

if "/opt/trn_rl_repo" not in sys.path:
    sys.path.insert(0, "/opt/trn_rl_repo")

import numpy as np

N, C, H, W = 32, 384, 56, 56
HW = H * W                      # 3136
M, D = 8, 768
N_CORES = 8
N_LOC = N // N_CORES            # 4 batch items per core
NM = N_LOC * M                  # 32 kv rows per core
D1P = 896                       # 768 + bias row, zero-padded to 7*128
KC = C // 128                   # 3 contraction chunks over c
P = 128

# hw subtiles (128 wide) for the softmax layout: 24 x 128 + 1 x 64
HWT = [128] * 24 + [64]
# macro groups of subtiles sharing one psum bank + one softmax pass
MACROS = [(0, 16), (16, 8), (24, 1)]
# scoresT hw tiles (one psum bank each)
HWT2 = [448] * 7
XA = 1792                       # x chunk split: [0,1792) + [1792,3136)

_cache = {}
last_results = None


def _build():
    from concourse import bacc, tile, mybir
    from concourse.masks import make_identity

    f32 = mybir.dt.float32
    f32r = mybir.dt.float32r
    Alu = mybir.AluOpType
    Act = mybir.ActivationFunctionType
    PSUM = tile.bass.MemorySpace.PSUM

    nc = bacc.Bacc("TRN2", target_bir_lowering=False, debug=False,
                   num_devices=N_CORES)

    xs_d = nc.dram_tensor("xs", [N_LOC, C, HW], f32r, kind="ExternalInput")
    gft_d = nc.dram_tensor("gft", [D1P, NM], f32r, kind="ExternalInput")
    wt_d = nc.dram_tensor("wt", [D1P, D], f32r, kind="ExternalInput")
    out_d = nc.dram_tensor("out", [N_LOC, C, HW], f32, kind="ExternalOutput")

    with tile.TileContext(nc) as tc:
        with tc.tile_pool(name="const", bufs=1) as const:
            ident = const.tile([P, P], f32, tag="ident")
            make_identity(nc, ident[:, :])
            identr = const.tile([P, P], f32r, tag="identr")
            nc.vector.tensor_copy(identr[:, :], ident[:, :])

            K_sb = const.tile([NM, C], f32r, tag="K_sb")
            V_n = [const.tile([M, C], f32r, tag=f"V{n}", name=f"V{n}")
                   for n in range(N_LOC)]
            KT = [const.tile([P, NM], f32r, tag=f"KT{kc}", name=f"KT{kc}")
                  for kc in range(KC)]

            with tc.tile_pool(name="wtp", bufs=1) as wtp, \
                 tc.tile_pool(name="psum0", bufs=1, space=PSUM) as psum0:
                wt_sb = []
                gft_sb = []
                for i in range(7):
                    w = wtp.tile([P, D], f32r, tag=f"wt{i}", name=f"wt{i}")
                    nc.sync.dma_start(w[:, :], wt_d.ap()[i * P:(i + 1) * P, :])
                    wt_sb.append(w)
                    g = const.tile([P, NM], f32r, tag=f"gft{i}", name=f"gft{i}")
                    nc.sync.dma_start(g[:, :],
                                      gft_d.ap()[i * P:(i + 1) * P, :])
                    gft_sb.append(g)
                kvK = psum0.tile([NM, C], f32, tag="kvK")
                for i in range(7):
                    nc.tensor.matmul(
                        kvK[:, :], gft_sb[i][:, :], wt_sb[i][:, :C],
                        start=(i == 0), stop=(i == 6))
                nc.vector.tensor_scalar(K_sb[:, :], kvK[:, :], 0.0, 6.0,
                                        op0=Alu.max, op1=Alu.min)
                # V per batch item at partition 0 (engine APs can't start at
                # partition 8/16/24), via lhsT free-dim slices of gft
                for n in range(N_LOC):
                    kvV = psum0.tile([M, C], f32, tag=f"kvV{n}",
                                     name=f"kvV{n}")
                    for i in range(7):
                        nc.tensor.matmul(
                            kvV[:, :], gft_sb[i][:, n * M:(n + 1) * M],
                            wt_sb[i][:, C:2 * C],
                            start=(i == 0), stop=(i == 6))
                    nc.vector.tensor_scalar(V_n[n][:, :], kvV[:, :],
                                            0.0, 6.0, op0=Alu.max, op1=Alu.min)
                for kc in range(KC):
                    ktp = psum0.tile([P, NM], f32r, tag="ktp")
                    nc.tensor.transpose(ktp[:, :],
                                        K_sb[:, kc * P:(kc + 1) * P],
                                        identr[:NM, :NM])
                    nc.scalar.copy(KT[kc][:, :], ktp[:, :])

            with (
                tc.tile_pool(name="xp", bufs=9) as xp,
                tc.tile_pool(name="sm", bufs=4) as sm,
                tc.tile_pool(name="sc8", bufs=1) as sc8,
                tc.tile_pool(name="aTp", bufs=3) as aTpool,
                tc.tile_pool(name="op", bufs=2) as op,
                tc.tile_pool(name="p8", bufs=3, space=PSUM) as p8,
                tc.tile_pool(name="ps_s", bufs=2, space=PSUM) as ps_s,
                tc.tile_pool(name="ps_o", bufs=3, space=PSUM) as ps_o,
            ):
                for n in range(N_LOC):
                    xc = []
                    for kc in range(KC):
                        ta = xp.tile([P, XA], f32r, tag="xa", name="xa")
                        nc.sync.dma_start(
                            ta[:, :], xs_d.ap()[n, kc * P:(kc + 1) * P, :XA])
                        tb = xp.tile([P, HW - XA], f32r, tag="xb", name="xb")
                        nc.sync.dma_start(
                            tb[:, :], xs_d.ap()[n, kc * P:(kc + 1) * P, XA:])
                        xc.append((ta, tb))

                    def xslice(kc, lo, w):
                        ta, tb = xc[kc]
                        if lo + w <= XA:
                            return ta[:, lo:lo + w]
                        return tb[:, lo - XA:lo - XA + w]

                    # mm1: scoresT[8, hw] tiles, x streaming at K=128
                    scTf = sc8.tile([M, HW], f32r, tag="scT_sb")
                    for t5, w5 in enumerate(HWT2):
                        pst = p8.tile([M, 512], f32, tag="b8", name="pst")
                        for kc in range(KC):
                            nc.tensor.matmul(
                                pst[:, :w5],
                                KT[kc][:, n * M:(n + 1) * M],
                                xslice(kc, t5 * 448, w5),
                                start=(kc == 0), stop=(kc == KC - 1))
                        nc.scalar.copy(scTf[:, t5 * 448:t5 * 448 + w5],
                                       pst[:, :w5])

                    aT = aTpool.tile([M, HW], f32r, tag="aT")

                    for ms, G in MACROS:
                        FD = M * G
                        pmac = HWT[ms]
                        ps = ps_s.tile([P, FD], f32r, tag="s")
                        # T1: scoresT 128-blocks -> scores[hw_p, m] slices
                        for jj in range(G):
                            j = ms + jj
                            pj = HWT[j]
                            nc.tensor.transpose(
                                ps[:pj, jj * M:(jj + 1) * M],
                                scTf[:, j * P:j * P + pj],
                                identr[:M, :M])

                        psf = ps[:pmac, :].bitcast(f32)
                        ps3 = psf.rearrange("p (g m) -> p g m", m=M)
                        nmx = sm.tile([P, G], f32, tag="nmx")
                        nc.vector.tensor_reduce(nmx[:pmac, :], ps3,
                                                axis=mybir.AxisListType.X,
                                                op=Alu.max, negate=True)
                        nmx_b = nmx[:pmac, :].unsqueeze(-1).broadcast_to(
                            [pmac, G, M])
                        e = sm.tile([P, FD], f32, tag="e")
                        e3 = e[:pmac, :].rearrange("p (g m) -> p g m", m=M)
                        nc.vector.tensor_add(e3, ps3, nmx_b)
                        nc.scalar.activation(e[:pmac, :], e[:pmac, :], Act.Exp)
                        den = sm.tile([P, G], f32, tag="den")
                        nc.vector.tensor_reduce(den[:pmac, :], e3,
                                                axis=mybir.AxisListType.X,
                                                op=Alu.add)
                        r = sm.tile([P, G], f32, tag="r")
                        nc.vector.reciprocal(r[:pmac, :], den[:pmac, :])
                        r_b = r[:pmac, :].unsqueeze(-1).broadcast_to(
                            [pmac, G, M])
                        attn = sm.tile([P, FD], f32r, tag="attn")
                        a3 = attn[:pmac, :].rearrange("p (g m) -> p g m", m=M)
                        nc.vector.tensor_mul(a3, e3, r_b)

                        # T2: attn subtiles -> attnT[8, hw], packed 4/bank
                        for pk in range(0, G, 4):
                            cnt = min(4, G - pk)
                            width = sum(HWT[ms + pk + q] for q in range(cnt))
                            pt = p8.tile([M, 512], f32r, tag="b8", name="pt")
                            for q in range(cnt):
                                jj = pk + q
                                pj = HWT[ms + jj]
                                nc.tensor.transpose(
                                    pt[:, q * P:q * P + pj],
                                    attn[:pj, jj * M:(jj + 1) * M],
                                    identr[:pj, :pj])
                            nc.scalar.copy(
                                aT[:, (ms + pk) * P:(ms + pk) * P + width],
                                pt[:, :width])

                    # mm2 + residual + store
                    for kc in range(KC):
                        osb = op.tile([P, HW], f32, tag="o")
                        for t7 in range(7):
                            po = ps_o.tile([P, 448], f32, tag="po")
                            nc.tensor.matmul(
                                po[:, :],
                                V_n[n][:, kc * P:(kc + 1) * P],
                                aT[:, t7 * 448:(t7 + 1) * 448],
                                start=True, stop=True)
                            nc.vector.tensor_add(
                                osb[:, t7 * 448:(t7 + 1) * 448], po[:, :],
                                xslice(kc, t7 * 448, 448).bitcast(f32))
                            if t7 == 3:
                                nc.gpsimd.dma_start(
                                    out_d.ap()[n, kc * P:(kc + 1) * P, :XA],
                                    osb[:, :XA])
                        nc.gpsimd.dma_start(
                            out_d.ap()[n, kc * P:(kc + 1) * P, XA:],
                            osb[:, XA:])

    nc.compile()
    return nc


def get_nc():
    if "nc" not in _cache:
        _cache["nc"] = _build()
    return _cache["nc"]


def make_in_maps(x, global_feature, W_kv, b_kv):
    x = np.ascontiguousarray(np.asarray(x, np.float32).reshape(N, C, HW))
    wt = np.zeros((D1P, D), np.float32)
    wt[:D] = np.asarray(W_kv, np.float32).T
    wt[D] = np.asarray(b_kv, np.float32)
    gf = np.asarray(global_feature, np.float32)
    in_maps = []
    for i in range(N_CORES):
        gfl = gf[i * N_LOC:(i + 1) * N_LOC].reshape(NM, D)
        gft = np.zeros((D1P, NM), np.float32)
        gft[:D] = gfl.T
        gft[D] = 1.0
        in_maps.append({
            "xs": np.ascontiguousarray(x[i * N_LOC:(i + 1) * N_LOC]),
            "gft": gft,
            "wt": wt,
        })
    return in_maps


def kernel(x, global_feature, W_kv, b_kv, trace=False):
    global last_results
    from concourse.bass_utils import run_bass_kernel_spmd

    nc = get_nc()
    in_maps = make_in_maps(x, global_feature, W_kv, b_kv)
    res = run_bass_kernel_spmd(nc, in_maps, core_ids=list(range(N_CORES)),
                               trace=trace)
    last_results = res
    out = np.concatenate([res.results[i]["out"][None] for i in range(N_CORES)],
                         axis=0)
    return out.reshape(N, C, H, W).astype(np.float32)


# revision 13
# speedup vs baseline: 1.1847x; 1.0554x over previous
"""Trainium2 Bass kernel for nn_Former_Mobile (mobile-former style cross-attention).

Computation (per batch item n):
    kv   = relu6(global_feature @ W_kv^T + b_kv)        # [m=8, 2c]
    K, V = kv[:, :c], kv[:, c:]                         # [8, c=384]
    q    = x reshaped [hw=3136, c]
    attn = softmax(q @ K^T)                             # [hw, 8]
    out  = (attn @ V) reshaped back + x                 # [c, hw]

Sharding: data-parallel over batch n across 8 NeuronCores (4 items each);
W_kv/b_kv replicated (bias folded into an extra contraction row host-side).

Matmul operands use float32r (PE relaxed-precision fp32: bf16-class speed,
~1e-4 relative rounding), accumulation in fp32 PSUM. Exact fp32 matmul on
TRN2 runs 4-8x slower per column (hi/lo dual pass at reduced rate), which
makes an fp32-exact kernel ~3x off the memory roofline; f32r recovers it.

Per-core device pipeline:
  phase 0: kv = gft-chunks @ wt-chunks (PE, psum accum) -> relu6 -> K^T via
           PE transpose (mm1 weights), per-n V rows (mm2 weights).
  per n (output phase software-pipelined one item behind the attention
  phase so the DVE-paced residual drain overlaps PE attention work):
    mm1   scoresT[8, hw-tile] = K^T(lhsT, 8 cols) @ x-chunk(rhs, K=128
          streaming), psum-accumulated over 3 c-chunks.
    T1    PE-transposes scoresT 128-blocks into scores[hw_p, m] psum macros
          (transpose-mode has fast weight load).
    softmax along free dim: DVE grouped reduce_max(negate) -> add broadcast
          -> ACT exp -> DVE grouped reduce_sum -> reciprocal -> mul.
    T2    PE-transposes attn tiles back into attnT[8, hw].
    mm2   out^T[c_p, hw-tile] = V(lhsT) @ attnT(rhs), single K=8 matmul.
    DVE residual add (psum + x -> sbuf), halved contiguous DMA out.
"""

import sys

if "/opt/trn_rl_repo" not in sys.path:
    sys.path.insert(0, "/opt/trn_rl_repo")

import numpy as np

N, C, H, W = 32, 384, 56, 56
HW = H * W                      # 3136
M, D = 8, 768
N_CORES = 8
N_LOC = N // N_CORES            # 4 batch items per core
NM = N_LOC * M                  # 32 kv rows per core
D1P = 896                       # 768 + bias row, zero-padded to 7*128
KC = C // 128                   # 3 contraction chunks over c
P = 128

# hw subtiles (128 wide) for the softmax layout: 24 x 128 + 1 x 64
HWT = [128] * 24 + [64]
# macro groups of subtiles sharing one psum bank + one softmax pass
MACROS = [(0, 16), (16, 9)]
# scoresT hw tiles (one psum bank each)
HWT2 = [448] * 7
XA = 1792                       # x chunk split: [0,1792) + [1792,3136)

_cache = {}
last_results = None


def _build():
    from concourse import bacc, tile, mybir
    from concourse.masks import make_identity

    f32 = mybir.dt.float32
    f32r = mybir.dt.float32r
    Alu = mybir.AluOpType
    Act = mybir.ActivationFunctionType
    PSUM = tile.bass.MemorySpace.PSUM

    nc = bacc.Bacc("TRN2", target_bir_lowering=False, debug=False,
                   num_devices=N_CORES)

    xs_d = nc.dram_tensor("xs", [N_LOC, C, HW], f32r, kind="ExternalInput")
    gft_d = nc.dram_tensor("gft", [D1P, NM], f32r, kind="ExternalInput")
    wt_d = nc.dram_tensor("wt", [D1P, D], f32r, kind="ExternalInput")
    out_d = nc.dram_tensor("out", [N_LOC, C, HW], f32, kind="ExternalOutput")

    with tile.TileContext(nc) as tc:
        with tc.tile_pool(name="const", bufs=1) as const:
            ident = const.tile([P, P], f32, tag="ident")
            make_identity(nc, ident[:, :])
            identr = const.tile([P, P], f32r, tag="identr")
            nc.vector.tensor_copy(identr[:, :], ident[:, :])

            K_sb = const.tile([NM, C], f32r, tag="K_sb")
            V_n = [const.tile([M, C], f32r, tag=f"V{n}", name=f"V{n}")
                   for n in range(N_LOC)]
            KT = [const.tile([P, NM], f32r, tag=f"KT{kc}", name=f"KT{kc}")
                  for kc in range(KC)]

            with tc.tile_pool(name="wtp", bufs=1) as wtp, \
                 tc.tile_pool(name="psum0", bufs=1, space=PSUM) as psum0:
                wt_sb = []
                gft_sb = []
                for i in range(7):
                    w = wtp.tile([P, D], f32r, tag=f"wt{i}", name=f"wt{i}")
                    nc.sync.dma_start(w[:, :], wt_d.ap()[i * P:(i + 1) * P, :])
                    wt_sb.append(w)
                    g = const.tile([P, NM], f32r, tag=f"gft{i}",
                                   name=f"gft{i}")
                    nc.sync.dma_start(g[:, :],
                                      gft_d.ap()[i * P:(i + 1) * P, :])
                    gft_sb.append(g)
                kvK = psum0.tile([NM, C], f32, tag="kvK")
                for i in range(7):
                    nc.tensor.matmul(
                        kvK[:, :], gft_sb[i][:, :], wt_sb[i][:, :C],
                        start=(i == 0), stop=(i == 6))
                nc.vector.tensor_scalar(K_sb[:, :], kvK[:, :], 0.0, 6.0,
                                        op0=Alu.max, op1=Alu.min)
                # V per batch item at partition 0 (engine APs can't start at
                # partition 8/16/24), via lhsT free-dim slices of gft
                for n in range(N_LOC):
                    kvV = psum0.tile([M, C], f32, tag=f"kvV{n}",
                                     name=f"kvV{n}")
                    for i in range(7):
                        nc.tensor.matmul(
                            kvV[:, :], gft_sb[i][:, n * M:(n + 1) * M],
                            wt_sb[i][:, C:2 * C],
                            start=(i == 0), stop=(i == 6))
                    nc.vector.tensor_scalar(V_n[n][:, :], kvV[:, :],
                                            0.0, 6.0, op0=Alu.max,
                                            op1=Alu.min)
                for kc in range(KC):
                    ktp = psum0.tile([P, NM], f32r, tag="ktp")
                    nc.tensor.transpose(ktp[:, :],
                                        K_sb[:, kc * P:(kc + 1) * P],
                                        identr[:NM, :NM])
                    nc.scalar.copy(KT[kc][:, :], ktp[:, :])

            with (
                tc.tile_pool(name="xp", bufs=9) as xp,
                tc.tile_pool(name="sm", bufs=4) as sm,
                tc.tile_pool(name="sc8", bufs=1) as sc8,
                tc.tile_pool(name="aTp", bufs=3) as aTpool,
                tc.tile_pool(name="op", bufs=2) as op,
                tc.tile_pool(name="p8", bufs=3, space=PSUM) as p8,
                tc.tile_pool(name="ps_s", bufs=2, space=PSUM) as ps_s,
                tc.tile_pool(name="ps_o", bufs=3, space=PSUM) as ps_o,
            ):
                def make_xslice(xc):
                    def xslice(kc, lo, w):
                        ta, tb = xc[kc]
                        if lo + w <= XA:
                            return ta[:, lo:lo + w]
                        return tb[:, lo - XA:lo - XA + w]
                    return xslice

                def gen_out(n, aT, xslice):
                    # mm2 + residual + store for item n; one t7 step per
                    # yield so it interleaves with the next item's attention
                    for kc in range(KC):
                        osb = op.tile([P, HW], f32, tag="o", name="osb")
                        for t7 in range(7):
                            po = ps_o.tile([P, 448], f32, tag="po", name="po")
                            nc.tensor.matmul(
                                po[:, :],
                                V_n[n][:, kc * P:(kc + 1) * P],
                                aT[:, t7 * 448:(t7 + 1) * 448],
                                start=True, stop=True)
                            nc.vector.tensor_add(
                                osb[:, t7 * 448:(t7 + 1) * 448], po[:, :],
                                xslice(kc, t7 * 448, 448).bitcast(f32))
                            if t7 == 3:
                                nc.gpsimd.dma_start(
                                    out_d.ap()[n, kc * P:(kc + 1) * P, :XA],
                                    osb[:, :XA])
                            yield
                        nc.gpsimd.dma_start(
                            out_d.ap()[n, kc * P:(kc + 1) * P, XA:],
                            osb[:, XA:])

                def drain(gen, steps):
                    if gen is None:
                        return None
                    try:
                        for _ in range(steps):
                            next(gen)
                    except StopIteration:
                        return None
                    return gen

                outgen = None
                for n in range(N_LOC):
                    xc = []
                    for kc in range(KC):
                        ta = xp.tile([P, XA], f32r, tag="xa", name="xa")
                        nc.sync.dma_start(
                            ta[:, :], xs_d.ap()[n, kc * P:(kc + 1) * P, :XA])
                        tb = xp.tile([P, HW - XA], f32r, tag="xb", name="xb")
                        nc.sync.dma_start(
                            tb[:, :], xs_d.ap()[n, kc * P:(kc + 1) * P, XA:])
                        xc.append((ta, tb))
                    xslice = make_xslice(xc)

                    # mm1: scoresT[8, hw] tiles, x streaming at K=128
                    scTf = sc8.tile([M, HW], f32r, tag="scT_sb")
                    for t5, w5 in enumerate(HWT2):
                        pst = p8.tile([M, 512], f32, tag="b8", name="pst")
                        for kc in range(KC):
                            nc.tensor.matmul(
                                pst[:, :w5],
                                KT[kc][:, n * M:(n + 1) * M],
                                xslice(kc, t5 * 448, w5),
                                start=(kc == 0), stop=(kc == KC - 1))
                        nc.scalar.copy(scTf[:, t5 * 448:t5 * 448 + w5],
                                       pst[:, :w5])
                        outgen = drain(outgen, 2)

                    aT = aTpool.tile([M, HW], f32r, tag="aT")

                    for ms, G in MACROS:
                        FD = M * G
                        ps = ps_s.tile([P, FD], f32r, tag="s")
                        # T1: scoresT 128-blocks -> scores[hw_p, m] slices
                        for jj in range(G):
                            j = ms + jj
                            pj = HWT[j]
                            nc.tensor.transpose(
                                ps[:pj, jj * M:(jj + 1) * M],
                                scTf[:, j * P:j * P + pj],
                                identr[:M, :M])
                        if ms + G - 1 == 24:
                            # last subtile is 64 rows; zero stale rows so the
                            # unused softmax lanes stay finite
                            nc.vector.memset(
                                ps[64:P, (G - 1) * M:G * M].bitcast(f32), 0.0)
                        outgen = drain(outgen, 2)

                        psf = ps[:, :].bitcast(f32)
                        ps3 = psf.rearrange("p (g m) -> p g m", m=M)
                        nmx = sm.tile([P, G], f32, tag="nmx")
                        nc.vector.tensor_reduce(nmx[:, :], ps3,
                                                axis=mybir.AxisListType.X,
                                                op=Alu.max, negate=True)
                        nmx_b = nmx[:, :].unsqueeze(-1).broadcast_to([P, G, M])
                        e = sm.tile([P, FD], f32, tag="e")
                        e3 = e[:, :].rearrange("p (g m) -> p g m", m=M)
                        nc.vector.tensor_add(e3, ps3, nmx_b)
                        nc.scalar.activation(e[:, :], e[:, :], Act.Exp)
                        den = sm.tile([P, G], f32, tag="den")
                        nc.vector.tensor_reduce(den[:, :], e3,
                                                axis=mybir.AxisListType.X,
                                                op=Alu.add)
                        r = sm.tile([P, G], f32, tag="r")
                        nc.vector.reciprocal(r[:, :], den[:, :])
                        r_b = r[:, :].unsqueeze(-1).broadcast_to([P, G, M])
                        attn = sm.tile([P, FD], f32r, tag="attn")
                        a3 = attn[:, :].rearrange("p (g m) -> p g m", m=M)
                        nc.vector.tensor_mul(a3, e3, r_b)
                        outgen = drain(outgen, 2)

                        # T2: attn subtiles -> attnT[8, hw], packed 4/bank
                        for pk in range(0, G, 4):
                            cnt = min(4, G - pk)
                            width = sum(HWT[ms + pk + q] for q in range(cnt))
                            pt = p8.tile([M, 512], f32r, tag="b8", name="pt")
                            for q in range(cnt):
                                jj = pk + q
                                pj = HWT[ms + jj]
                                nc.tensor.transpose(
                                    pt[:, q * P:q * P + pj],
                                    attn[:pj, jj * M:(jj + 1) * M],
                                    identr[:pj, :pj])
                            nc.scalar.copy(
                                aT[:, (ms + pk) * P:(ms + pk) * P + width],
                                pt[:, :width])
                            outgen = drain(outgen, 2)

                    # flush the previous item's output phase, then queue ours
                    while outgen is not None:
                        outgen = drain(outgen, 4)
                    outgen = gen_out(n, aT, xslice)
                while outgen is not None:
                    outgen = drain(outgen, 4)

    nc.compile()
    return nc


def get_nc():
    if "nc" not in _cache:
        _cache["nc"] = _build()
    return _cache["nc"]


def make_in_maps(x, global_feature, W_kv, b_kv):
    x = np.ascontiguousarray(np.asarray(x, np.float32).reshape(N, C, HW))
    wt = np.zeros((D1P, D), np.float32)
    wt[:D] = np.asarray(W_kv, np.float32).T
    wt[D] = np.asarray(b_kv, np.float32)
    gf = np.asarray(global_feature, np.float32)
    in_maps = []
    for i in range(N_CORES):
        gfl = gf[i * N_LOC:(i + 1) * N_LOC].reshape(NM, D)
        gft = np.zeros((D1P, NM), np.float32)
        gft[:D] = gfl.T
        gft[D] = 1.0
        in_maps.append({
            "xs": np.ascontiguousarray(x[i * N_LOC:(i + 1) * N_LOC]),
            "gft": gft,
            "wt": wt,
        })
    return in_maps


def kernel(x, global_feature, W_kv, b_kv, trace=False):
    global last_results
    from concourse.bass_utils import run_bass_kernel_spmd

    nc = get_nc()
    in_maps = make_in_maps(x, global_feature, W_kv, b_kv)
    res = run_bass_kernel_spmd(nc, in_maps, core_ids=list(range(N_CORES)),
                               trace=trace)
    last_results = res
    out = np.concatenate([res.results[i]["out"][None] for i in range(N_CORES)],
                         axis=0)
    return out.reshape(N, C, H, W).astype(np.float32)
